# revision 1
# baseline (speedup 1.0000x reference)
"""CARAFE content-aware upsampling kernel for Trainium2 (Bass/Tile), 8 NeuronCores.

Problem (hardcoded): features [4, 256, 64, 64] f32, masks [4, 25, 128, 128] f32,
K=5, G=1, S=2 -> output [4, 256, 128, 128] f32.

Strategy
--------
Sharding: 8 cores = (batch n in 0..3) x (output-row half yh in 0..1); each core
computes out[n, :, yh*64:(yh+1)*64, :] for ALL 256 channels. The banded mask
operand depends only on (n, y), so splitting y (not channels) halves its HBM
traffic.

Compute mapping: CARAFE's per-output-pixel 25-tap weighted sum is cast as
TensorEngine matmuls contracting over the padded input-width axis wp (K=68):

  out[c, 2*h2+py, x] = sum_{hp, wp} bnd[hp, wp, kr=hp-h2, py, x] * ft[wp, hp, c]

ft is the zero-padded transposed feature map (bf16); bnd is a host-built banded
mask operand (bf16): for input row hp and tap-row kr, column (py, x) holds
mask m[kr*5+dw, 2*(hp-kr)+py, x] at partition wp = floor(x/2)+dw, else zero.

Per local input row hpl (36 rows/core): 2 stationary loads (c-halves) and up to
10 matmuls ([68, 256] moving operand) accumulating f32 into the PSUM tile
[128, 512] of output pair h2 = hpl-kr (c-half 0 in cols 0:256, half 1 in
256:512; one accumulation group per tile since PSUM zero regions are
bank-granular). A rolling window of 5 PSUM tiles stays live; completed pairs
are copied (cast to bf16) into SBUF staging on alternating DVE/ACT engines and
DMA'd out two pairs at a time on alternating SP/ACT HWDGE rings; the host
upcasts to f32.
"""

import sys

sys.path.insert(0, "/opt/trn_rl_repo")

import numpy as np
import ml_dtypes

import concourse.bacc as bacc
import concourse.mybir as mybir
from concourse import tile
from concourse import bass_utils

N, C, H, W = 4, 256, 64, 64
S = 2
KK = 5
HO, WO = H * S, W * S  # 128, 128
HP = H + KK - 1  # 68 padded rows
WP = W + KK - 1  # 68 padded cols
NCORES = 8

HPL = 36  # padded input rows per core (32 pairs + 4 tap overlap)
NPAIR = 32  # output row-pairs per core
NBLK = 18  # hpl DMA blocks of 2
BLKH = 2  # hpl rows per band DMA block
FW = KK * 2 * WO  # 1280 band cols per input row

BF16 = ml_dtypes.bfloat16


def _host_prep(features: np.ndarray, masks: np.ndarray):
    """Per-core transposed/padded features and banded mask operands."""
    # ft_g[n, wp, hp, c] = features[n, c, hp-2, wp-2]  (zero pad)
    ft_g = np.zeros((N, WP, HP, C), np.float32)
    ft_g[:, 2 : 2 + W, 2 : 2 + H, :] = features.transpose(0, 3, 2, 1)
    ft_g = ft_g.astype(BF16)

    # bnd_g[n, hp, wp, kr, py, x] = masks[n, kr*5+dw, 2*(hp-kr)+py, x]
    #   where dw = wp - floor(x/2), nonzero only for dw in [0, 5)
    bnd_g = np.zeros((N, HP, WP, KK, 2, WO), np.float32)
    st = [s // 4 for s in bnd_g.strides]  # element strides
    m6 = masks.reshape(N, KK * KK, H, 2, W, 2)
    for kr in range(KK):
        for dw in range(KK):
            base = bnd_g[:, kr:, dw:, kr, :, :]
            view = np.lib.stride_tricks.as_strided(
                base,
                shape=(N, H, 2, W, 2),
                strides=tuple(
                    4 * s
                    for s in (st[0], st[1], st[4], st[2] + 2 * st[5], st[5])
                ),
            )
            view[...] = m6[:, kr * KK + dw]
    bnd_g = bnd_g.astype(BF16)

    fts, bnds = [], []
    for i in range(NCORES):
        n, yh = divmod(i, 2)
        fts.append(np.ascontiguousarray(ft_g[n, :, yh * NPAIR : yh * NPAIR + HPL, :]))
        b = bnd_g[n, yh * NPAIR : yh * NPAIR + HPL].reshape(NBLK, BLKH, WP, FW)
        bnds.append(np.ascontiguousarray(b.transpose(0, 2, 1, 3)))
    return fts, bnds


_NC_CACHE = []


def _build_nc():
    """Build + compile the single-core Tile program (same for all 8 cores)."""
    if _NC_CACHE:
        return _NC_CACHE[0]

    nc = bacc.Bacc("TRN2", target_bir_lowering=False, debug=False)
    ft = nc.dram_tensor(
        "ft", [WP, HPL * C], mybir.dt.bfloat16, kind="ExternalInput"
    ).ap()
    bnd = nc.dram_tensor(
        "bnd", [NBLK, WP, BLKH * FW], mybir.dt.bfloat16, kind="ExternalInput"
    ).ap()
    out = nc.dram_tensor(
        "out", [C, 2 * NPAIR, WO], mybir.dt.bfloat16, kind="ExternalOutput"
    ).ap()
    outf = out.rearrange("c y x -> c (y x)")  # [256, 64*128]

    with tile.TileContext(nc) as tc:
        with (
            tc.tile_pool(name="ftp", bufs=4) as ftp,
            tc.tile_pool(name="bnp", bufs=6) as bnp,
            tc.tile_pool(name="pp", bufs=8, space="PSUM") as pp,
            tc.tile_pool(name="op", bufs=4) as op,
        ):
            psums = {}
            ft_tiles = {}
            FC = 9  # hpl rows per feature chunk tile
            for blk in range(NBLK):
                bnt = bnp.tile([WP, BLKH * FW], mybir.dt.bfloat16)
                nc.sync.dma_start(bnt[:], bnd[blk])
                if blk in (0, 2, 4, 6):
                    # interleave feature chunks between the band blocks
                    ci = blk // 2
                    fct = ftp.tile([WP, FC * C], mybir.dt.bfloat16,
                                   name="fct", tag="fct")
                    nc.scalar.dma_start(
                        fct[:], ft[:, ci * FC * C : (ci + 1) * FC * C]
                    )
                    ft_tiles[ci] = fct
                for i4 in range(BLKH):
                    hpl = BLKH * blk + i4
                    for ch in (0, 1):
                        fci, fcr = divmod(hpl, FC)
                        lhsT = ft_tiles[fci][
                            :, fcr * C + ch * 128 : fcr * C + ch * 128 + 128
                        ]
                        for kr in range(KK):
                            h2 = hpl - kr
                            if not (0 <= h2 < NPAIR):
                                continue
                            if kr == 0 and ch == 0:
                                psums[h2] = pp.tile(
                                    [128, 2 * 2 * WO], mybir.dt.float32,
                                    name="ps", tag="ps",
                                )
                            # One PSUM accumulation group per pair tile (zero
                            # regions are bank-granular): open at the first
                            # matmul (ch0/kr0), close at the last (ch1/kr4).
                            nc.tensor.matmul(
                                psums[h2][:, ch * 2 * WO : (ch + 1) * 2 * WO],
                                lhsT,
                                bnt[:, i4 * FW + kr * 2 * WO : i4 * FW + (kr + 1) * 2 * WO],
                                start=(kr == 0 and ch == 0),
                                stop=(kr == KK - 1 and ch == 1),
                            )
                    h2 = hpl - (KK - 1)
                    if 0 <= h2 < NPAIR:
                        pt = psums.pop(h2)
                        g = h2 % 2
                        if g == 0:
                            ot = op.tile([128, 2 * 512], mybir.dt.bfloat16,
                                         name="ot", tag="ot")
                            psums["ot"] = ot
                        ot = psums["ot"]
                        # staging cols: [ch, g, py*x] to keep DMA APs 3-dim
                        otv = ot.rearrange("p (ch g f) -> p ch g f", ch=2, g=2)
                        src = pt.rearrange("p (ch f) -> p ch f", ch=2)
                        if (h2 // 2) % 2 == 0:
                            nc.vector.tensor_copy(otv[:, :, g, :], src)
                        else:
                            nc.scalar.copy(otv[:, :, g, :], src)
                        if g == 1:
                            sv = ot.rearrange("p (ch gf) -> p ch gf", ch=2)
                            ov = outf.rearrange("(ch p) f -> p ch f", ch=2)
                            g0 = h2 - 1
                            deng = nc.scalar if (h2 // 2) % 2 == 0 else nc.sync
                            deng.dma_start(
                                ov[:, :, 2 * WO * g0 : 2 * WO * (g0 + 2)], sv
                            )

    nc.compile()
    _NC_CACHE.append(nc)
    return nc


def kernel(features: np.ndarray, masks: np.ndarray) -> np.ndarray:
    features = np.ascontiguousarray(features, dtype=np.float32)
    masks = np.ascontiguousarray(masks, dtype=np.float32)
    fts, bnds = _host_prep(features, masks)

    nc = _build_nc()
    in_maps = [
        {"ft": fts[i].reshape(WP, HPL * C), "bnd": bnds[i].reshape(NBLK, WP, BLKH * FW)}
        for i in range(NCORES)
    ]

    res = bass_utils.run_bass_kernel_spmd(nc, in_maps, list(range(NCORES)))

    out = np.empty((N, C, HO, WO), np.float32)
    for i in range(NCORES):
        n, yh = divmod(i, 2)
        out[n, :, yh * 2 * NPAIR : (yh + 1) * 2 * NPAIR, :] = (
            res.results[i]["out"].astype(np.float32).reshape(C, 2 * NPAIR, WO)
        )
    return out



# revision 2
# speedup vs baseline: 1.5289x; 1.5289x over previous
"""CARAFE content-aware upsampling kernel for Trainium2 (Bass/Tile), 8 NeuronCores.

Problem (hardcoded): features [4, 256, 64, 64] f32, masks [4, 25, 128, 128] f32,
K=5, G=1, S=2 -> output [4, 256, 128, 128] f32.

Strategy
--------
Sharding: 8 cores = (batch n in 0..3) x (output-row half yh in 0..1); each core
computes out[n, :, yh*64:(yh+1)*64, :] for ALL 256 channels.

Compute mapping: each output block of (4 row-pairs x 16 columns) = 128 output
positions depends on an 8-row x 12-col window of the padded input feature map.
Flattening that window gives a 96-long contraction axis that covers ALL 25
CARAFE taps in a single matmul:

  out[c, pos] = sum_k ftr[k, c] * bnd[k, pos],   k = (hpw, wpw) in 8 x 12

ftr is the host-replicated feature window per block (bf16); bnd is a
host-built banded mask operand (bf16): bnd[(hpw,wpw), (p4,py,xl)] =
mask[kr*5+dw, y, x] with kr = hpw-p4, dw = wpw-xl//2 when both fall in [0,5),
else zero. One matmul per (block, channel-half): 128 matmuls of 128 moving
columns each per core (~16k PE cycles), PSUM output lands directly in
[c, y-major] layout so no transpose is needed anywhere.

Dataflow: ftr and bnd are packed per row-group chunk into one DRAM tensor and
streamed with 5 big DMAs; per row-group a [128, 2048] PSUM tile (4 banks)
collects 16 independent matmuls (start=stop=True each, disjoint columns);
DVE and ACT each cast one channel-half to bf16 into a shared staging tile
(reordering to y-major), and one DMA per row-group writes [256ch x 8row x 128col]
to DRAM. The host upcasts to f32.
"""

import sys

sys.path.insert(0, "/opt/trn_rl_repo")

import numpy as np
import ml_dtypes

import concourse.bacc as bacc
import concourse.mybir as mybir
from concourse import tile
from concourse import bass_utils

N, C, H, W = 4, 256, 64, 64
KK = 5
HO, WO = 128, 128
NCORES = 8

HPL = 36          # padded input rows per core (32 pairs + 4 tap overlap)
WP = 68           # padded input cols
NHG = 8           # row-groups per core (4 row-pairs each)
NXB = 8           # col-blocks per core (16 output cols each)
KDIM = 96         # contraction: 8 hp x 12 wp
CHUNKS = (1, 1, 2, 2, 2)   # hgrps per input DMA chunk
HGW = 8 * (256 + 128)      # fbd cols per hgrp: 8 tiles x (ftr 256 + bnd 128)

BF16 = ml_dtypes.bfloat16


def _host_prep(features: np.ndarray, masks: np.ndarray):
    """Per-core packed (ftr || bnd) chunk operand, [96, 24576] bf16."""
    featT = features.transpose(0, 2, 3, 1)  # [N, H, W, C]
    fbds = []
    for i in range(NCORES):
        n, yh = divmod(i, 2)
        # padded transposed features: feat_pad[hp, wp, c] = features[n, c, yh*32+hp-2, wp-2]
        feat_pad = np.zeros((HPL, WP, C), np.float32)
        r0 = yh * 32 - 2
        lo, hi = max(0, -r0), min(HPL, H - r0)
        feat_pad[lo:hi, 2:2 + W, :] = featT[n, r0 + lo:r0 + hi]

        # ftr[k=(hpw*12+wpw), t=(hgrp*8+xblk), c]
        s_hp, s_wp, s_c = feat_pad.strides
        ftr = np.lib.stride_tricks.as_strided(
            feat_pad,
            shape=(8, 12, NHG, NXB, C),
            strides=(s_hp, s_wp, 4 * s_hp, 8 * s_wp, s_c),
        ).reshape(KDIM, NHG, 8 * C)

        # bnd[k, (hgrp, xblk), (p4, py, xl)]
        ml = masks[n, :, yh * 64:(yh + 1) * 64, :]  # [25, 64, 128]
        bnd = np.zeros((8, 12, NHG, NXB, 4, 2, 16), np.float32)
        s = bnd.strides
        for kr in range(KK):
            for dw in range(KK):
                # dest dims (p4, hgrp, xblk, xw, py, q):
                #   bnd[p4+kr, dw+xw, hgrp, xblk, p4, py, 2*xw+q]
                dv = np.lib.stride_tricks.as_strided(
                    bnd[kr, dw],
                    shape=(4, NHG, NXB, 8, 2, 2),
                    strides=(s[0] + s[4], s[2], s[3], s[1] + 2 * s[6], s[5], s[6]),
                )
                sv = ml[kr * KK + dw].reshape(NHG, 4, 2, NXB, 8, 2)
                dv[...] = sv.transpose(1, 0, 3, 4, 2, 5)
        bnd = bnd.reshape(KDIM, NHG, 8 * 128)

        fbd = np.empty((KDIM, NHG * HGW), np.float32)
        off = 0
        h0 = 0
        for g in CHUNKS:
            fw, bw = g * 8 * C, g * 8 * 128
            fbd[:, off:off + fw] = ftr[:, h0:h0 + g].reshape(KDIM, fw)
            fbd[:, off + fw:off + fw + bw] = bnd[:, h0:h0 + g].reshape(KDIM, bw)
            off += fw + bw
            h0 += g
        fbds.append(fbd.astype(BF16))
    return fbds


_NC_CACHE = []


def _build_nc():
    """Build + compile the single-core Tile program (same for all 8 cores)."""
    if _NC_CACHE:
        return _NC_CACHE[0]

    nc = bacc.Bacc("TRN2", target_bir_lowering=False, debug=False)
    fbd = nc.dram_tensor(
        "fbd", [KDIM, NHG * HGW], mybir.dt.bfloat16, kind="ExternalInput"
    ).ap()
    out = nc.dram_tensor(
        "out", [C, 64 * 128], mybir.dt.bfloat16, kind="ExternalOutput"
    ).ap()
    # out view [ch, c, hgrp, f=1024]
    ov = out.rearrange("(ch c) (hgrp f) -> ch c hgrp f", ch=2, hgrp=NHG)

    with tile.TileContext(nc) as tc:
        with (
            tc.tile_pool(name="fbp", bufs=len(CHUNKS)) as fbp,
            tc.tile_pool(name="stp", bufs=3) as stp,
            tc.tile_pool(name="pp", bufs=2, space="PSUM") as pp,
        ):
            chunk_of = []   # per hgrp: (tile, local hgrp index, group size)
            off = 0
            for g in CHUNKS:
                t = fbp.tile([KDIM, g * HGW], mybir.dt.bfloat16, name="fb", tag="fb")
                nc.sync.dma_start(t[:], fbd[:, off:off + g * HGW])
                off += g * HGW
                for hh in range(g):
                    chunk_of.append((t, hh, g))

            for hgrp in range(NHG):
                fb, hh, g = chunk_of[hgrp]
                foff = hh * 8 * C
                boff = g * 8 * C + hh * 8 * 128
                ps = pp.tile([128, 2048], mybir.dt.float32, name="ps", tag="ps")
                for xblk in range(NXB):
                    rhs = fb[:, boff + xblk * 128: boff + (xblk + 1) * 128]
                    for ch in range(2):
                        lhsT = fb[:, foff + xblk * 256 + ch * 128:
                                   foff + xblk * 256 + (ch + 1) * 128]
                        nc.tensor.matmul(
                            ps[:, (xblk * 2 + ch) * 128: (xblk * 2 + ch + 1) * 128],
                            lhsT,
                            rhs,
                            start=True,
                            stop=True,
                        )
                st = stp.tile([128, 2048], mybir.dt.bfloat16, name="st", tag="st")
                # psum cols (xblk, ch, p4, py, xl) -> staging cols (ch, p4, py, xblk, xl)
                sv = ps.rearrange(
                    "c (xblk ch p4 py xl) -> c ch xblk p4 py xl",
                    xblk=8, ch=2, p4=4, py=2,
                )
                dv = st.rearrange(
                    "c (ch p4 py xblk xl) -> c ch xblk p4 py xl",
                    ch=2, p4=4, py=2, xblk=8,
                )
                nc.vector.tensor_copy(dv[:, 0], sv[:, 0])
                nc.scalar.copy(dv[:, 1], sv[:, 1])
                # staging [c, (ch f)] -> out[ch*128+c, hgrp*1024 + f]
                sov = st.rearrange("c (ch f) -> c ch f", ch=2)
                nc.sync.dma_start(ov[:, :, hgrp, :].rearrange("ch c f -> c ch f"), sov)

    nc.compile()
    _NC_CACHE.append(nc)
    return nc


def kernel(features: np.ndarray, masks: np.ndarray) -> np.ndarray:
    features = np.ascontiguousarray(features, dtype=np.float32)
    masks = np.ascontiguousarray(masks, dtype=np.float32)
    fbds = _host_prep(features, masks)

    nc = _build_nc()
    in_maps = [{"fbd": fbds[i]} for i in range(NCORES)]

    res = bass_utils.run_bass_kernel_spmd(nc, in_maps, list(range(NCORES)))

    out = np.empty((N, C, HO, WO), np.float32)
    for i in range(NCORES):
        n, yh = divmod(i, 2)
        out[n, :, yh * 64:(yh + 1) * 64, :] = (
            res.results[i]["out"].astype(np.float32).reshape(C, 64, 128)
        )
    return out


# revision 3
# speedup vs baseline: 1.7492x; 1.1440x over previous
"""CARAFE content-aware upsampling kernel for Trainium2 (Bass/Tile), 8 NeuronCores.

Problem (hardcoded): features [4, 256, 64, 64] f32, masks [4, 25, 128, 128] f32,
K=5, G=1, S=2 -> output [4, 256, 128, 128] f32.

Strategy
--------
Sharding: 8 cores = (batch n in 0..3) x (output-row half yh in 0..1); each core
computes out[n, :, yh*64:(yh+1)*64, :] for ALL 256 channels.

Compute mapping: each output block of (4 row-pairs x 16 columns) = 128 output
positions depends on an 8-row x 12-col window of the padded input feature map.
Flattening that window gives a 96-long contraction axis that covers ALL 25
CARAFE taps in a single matmul:

  out[c, pos] = sum_k ftr[k, c] * bnd[k, pos],   k = (hpw, wpw) in 8 x 12

ftr is the host-replicated feature window per block (bf16); bnd is a
host-built banded mask operand (bf16): bnd[(hpw,wpw), (p4,py,xl)] =
mask[kr*5+dw, y, x] with kr = hpw-p4, dw = wpw-xl//2 when both fall in [0,5),
else zero. One matmul per (block, channel-half): 128 matmuls of 128 moving
columns each per core (~16k PE cycles), PSUM output lands directly in
[c, y-major] layout so no transpose is needed anywhere.

Dataflow: ftr and bnd are packed per row-group chunk into one DRAM tensor and
streamed with 5 big DMAs; per row-group a [128, 2048] PSUM tile (4 banks)
collects 16 independent matmuls (start=stop=True each, disjoint columns);
DVE and ACT each cast one channel-half to bf16 into a shared staging tile
(reordering to y-major), and one DMA per row-group writes [256ch x 8row x 128col]
to DRAM. The host upcasts to f32.
"""

import sys

sys.path.insert(0, "/opt/trn_rl_repo")

import numpy as np
import ml_dtypes

import concourse.bacc as bacc
import concourse.mybir as mybir
from concourse import tile
from concourse import bass_utils

N, C, H, W = 4, 256, 64, 64
KK = 5
HO, WO = 128, 128
NCORES = 8

HPL = 36          # padded input rows per core (32 pairs + 4 tap overlap)
WP = 68           # padded input cols
NHG = 8           # row-groups per core (4 row-pairs each)
NXB = 8           # col-blocks per core (16 output cols each)
KDIM = 96         # contraction: 8 hp x 12 wp
CHUNKS = (1, 1, 2, 2, 2)   # hgrps per input DMA chunk
HGW = 8 * (256 + 128)      # fbd cols per hgrp: 8 tiles x (ftr 256 + bnd 128)

BF16 = ml_dtypes.bfloat16


def _host_prep(features: np.ndarray, masks: np.ndarray):
    """Per-core packed (ftr || bnd) chunk operand, [96, 24576] bf16."""
    featT = features.transpose(0, 2, 3, 1)  # [N, H, W, C]
    fbds = []
    for i in range(NCORES):
        n, yh = divmod(i, 2)
        # padded transposed features: feat_pad[hp, wp, c] = features[n, c, yh*32+hp-2, wp-2]
        feat_pad = np.zeros((HPL, WP, C), np.float32)
        r0 = yh * 32 - 2
        lo, hi = max(0, -r0), min(HPL, H - r0)
        feat_pad[lo:hi, 2:2 + W, :] = featT[n, r0 + lo:r0 + hi]

        # ftr[k=(hpw*12+wpw), t=(hgrp*8+xblk), c]
        s_hp, s_wp, s_c = feat_pad.strides
        ftr = np.lib.stride_tricks.as_strided(
            feat_pad,
            shape=(8, 12, NHG, NXB, C),
            strides=(s_hp, s_wp, 4 * s_hp, 8 * s_wp, s_c),
        ).reshape(KDIM, NHG, 8 * C)

        # bnd[k, (hgrp, xblk), (p4, py, xl)]
        ml = masks[n, :, yh * 64:(yh + 1) * 64, :]  # [25, 64, 128]
        bnd = np.zeros((8, 12, NHG, NXB, 4, 2, 16), np.float32)
        s = bnd.strides
        for kr in range(KK):
            for dw in range(KK):
                # dest dims (p4, hgrp, xblk, xw, py, q):
                #   bnd[p4+kr, dw+xw, hgrp, xblk, p4, py, 2*xw+q]
                dv = np.lib.stride_tricks.as_strided(
                    bnd[kr, dw],
                    shape=(4, NHG, NXB, 8, 2, 2),
                    strides=(s[0] + s[4], s[2], s[3], s[1] + 2 * s[6], s[5], s[6]),
                )
                sv = ml[kr * KK + dw].reshape(NHG, 4, 2, NXB, 8, 2)
                dv[...] = sv.transpose(1, 0, 3, 4, 2, 5)
        bnd = bnd.reshape(KDIM, NHG, 8 * 128)

        fbd = np.empty((KDIM, NHG * HGW), np.float32)
        off = 0
        h0 = 0
        for g in CHUNKS:
            fw, bw = g * 8 * C, g * 8 * 128
            fbd[:, off:off + fw] = ftr[:, h0:h0 + g].reshape(KDIM, fw)
            fbd[:, off + fw:off + fw + bw] = bnd[:, h0:h0 + g].reshape(KDIM, bw)
            off += fw + bw
            h0 += g
        fbds.append(fbd.astype(BF16))
    return fbds


_NC_CACHE = []


def _build_nc():
    """Build + compile the single-core Tile program (same for all 8 cores)."""
    if _NC_CACHE:
        return _NC_CACHE[0]

    nc = bacc.Bacc("TRN2", target_bir_lowering=False, debug=False)
    fbd = nc.dram_tensor(
        "fbd", [KDIM, NHG * HGW], mybir.dt.bfloat16, kind="ExternalInput"
    ).ap()
    out = nc.dram_tensor(
        "out", [C, 64 * 128], mybir.dt.bfloat16, kind="ExternalOutput"
    ).ap()
    # out view [ch, c, hgrp, f=1024]
    ov = out.rearrange("(ch c) (hgrp f) -> ch c hgrp f", ch=2, hgrp=NHG)

    with tile.TileContext(nc) as tc:
        with (
            tc.tile_pool(name="fbp", bufs=len(CHUNKS)) as fbp,
            tc.tile_pool(name="stp", bufs=8) as stp,
            tc.tile_pool(name="pp", bufs=2, space="PSUM") as pp,
        ):
            chunk_of = []   # per hgrp: (tile, local hgrp index, group size)
            off = 0
            for g in CHUNKS:
                t = fbp.tile([KDIM, g * HGW], mybir.dt.bfloat16, name="fb", tag="fb")
                nc.sync.dma_start(t[:], fbd[:, off:off + g * HGW])
                off += g * HGW
                for hh in range(g):
                    chunk_of.append((t, hh, g))

            for hgrp in range(NHG):
                fb, hh, g = chunk_of[hgrp]
                foff = hh * 8 * C
                boff = g * 8 * C + hh * 8 * 128
                ps = pp.tile([128, 2048], mybir.dt.float32, name="ps", tag="ps")
                for xblk in range(NXB):
                    rhs = fb[:, boff + xblk * 128: boff + (xblk + 1) * 128]
                    for ch in range(2):
                        lhsT = fb[:, foff + xblk * 256 + ch * 128:
                                   foff + xblk * 256 + (ch + 1) * 128]
                        nc.tensor.matmul(
                            ps[:, (xblk * 2 + ch) * 128: (xblk * 2 + ch + 1) * 128],
                            lhsT,
                            rhs,
                            start=True,
                            stop=True,
                        )
                st = stp.tile([128, 2048], mybir.dt.bfloat16, name="st", tag="st")
                # psum cols (xblk, ch, p4, py, xl) -> staging cols (ch, p4, py, xblk, xl)
                sv = ps.rearrange(
                    "c (xblk ch p4 py xl) -> c ch xblk p4 py xl",
                    xblk=8, ch=2, p4=4, py=2,
                )
                dv = st.rearrange(
                    "c (ch p4 py xblk xl) -> c ch xblk p4 py xl",
                    ch=2, p4=4, py=2, xblk=8,
                )
                nc.vector.tensor_copy(dv[:, 0], sv[:, 0])
                nc.scalar.copy(dv[:, 1], sv[:, 1])
                # staging [c, (ch f)] -> out[ch*128+c, hgrp*1024 + f]
                sov = st.rearrange("c (ch f) -> c ch f", ch=2)
                nc.sync.dma_start(ov[:, :, hgrp, :].rearrange("ch c f -> c ch f"), sov)

    nc.compile()
    _NC_CACHE.append(nc)
    return nc


def kernel(features: np.ndarray, masks: np.ndarray) -> np.ndarray:
    features = np.ascontiguousarray(features, dtype=np.float32)
    masks = np.ascontiguousarray(masks, dtype=np.float32)
    fbds = _host_prep(features, masks)

    nc = _build_nc()
    in_maps = [{"fbd": fbds[i]} for i in range(NCORES)]

    res = bass_utils.run_bass_kernel_spmd(nc, in_maps, list(range(NCORES)))

    out = np.empty((N, C, HO, WO), np.float32)
    for i in range(NCORES):
        n, yh = divmod(i, 2)
        out[n, :, yh * 64:(yh + 1) * 64, :] = (
            res.results[i]["out"].astype(np.float32).reshape(C, 64, 128)
        )
    return out


# revision 5
# speedup vs baseline: 1.7872x; 1.0218x over previous
"""CARAFE content-aware upsampling kernel for Trainium2 (Bass/Tile), 8 NeuronCores.

Problem (hardcoded): features [4, 256, 64, 64] f32, masks [4, 25, 128, 128] f32,
K=5, G=1, S=2 -> output [4, 256, 128, 128] f32.

Strategy
--------
Sharding: 8 cores = (batch n in 0..3) x (output-row half yh in 0..1); each core
computes out[n, :, yh*64:(yh+1)*64, :] for ALL 256 channels.

Compute mapping: each output block of (4 row-pairs x 16 columns) = 128 output
positions depends on an 8-row x 12-col window of the padded input feature map.
Flattening that window gives a 96-long contraction axis (k = wpw*12.. actually
k = wpw*8 + hpw) that covers ALL 25 CARAFE taps in a single matmul:

  out[c, pos] = sum_k ftr[k, c] * bnd[k, pos],   k = (wpw, hpw) in 12 x 8

ftr is the host-replicated feature window per block (bf16); bnd is a
host-built banded mask operand (bf16): bnd[(wpw,hpw), (p4,py,xl)] =
mask[kr*5+dw, y, x] with kr = hpw-p4, dw = wpw-xl//2 when both fall in [0,5),
else zero. One matmul per (block, channel-half): 128 matmuls of 128 moving
columns each per core (~16k PE cycles), and the PSUM output lands directly in
[c, y-major] layout so no transpose is needed anywhere.

Edge columns: for xblk 0 the contraction rows wpw in {0,1} hit zero-padded
feature columns (wp in {0,1}); for xblk 7 the rows wpw in {10,11} do. With the
wpw-major row order those rows are contiguous ([0,16) resp. [80,96)), so edge
tiles ship only 80 contraction rows and the matmul contracts the sub-range.

Dataflow: ftr and bnd are packed per row-group chunk into one DRAM tensor
(full-K block + two edge blocks) and streamed with 3 DMAs per chunk; per
row-group a [128, 2048] PSUM tile (4 banks) collects 16 independent matmuls
(start=stop=True each, disjoint columns); DVE and ACT each cast one
channel-half to bf16 into a shared staging tile (reordering to y-major), and
one DMA per row-group writes [256ch x 8row x 128col] to DRAM. The host
upcasts to f32.
"""

import sys

sys.path.insert(0, "/opt/trn_rl_repo")

import numpy as np
import ml_dtypes

import concourse.bacc as bacc
import concourse.mybir as mybir
from concourse import tile
from concourse import bass_utils

N, C, H, W = 4, 256, 64, 64
KK = 5
HO, WO = 128, 128
NCORES = 8

HPL = 36          # padded input rows per core (32 pairs + 4 tap overlap)
WP = 68           # padded input cols
NHG = 8           # row-groups per core (4 row-pairs each)
NXB = 8           # col-blocks per core (16 output cols each)
KDIM = 96         # contraction: 12 wp x 8 hp
KE = 80           # contraction rows for edge col-blocks (xblk 0 and 7)
CHUNKS = (1, 1, 2, 2, 2)   # hgrps per input DMA chunk

# per-hgrp fbd column layout: mid block (xblk 1..6, 96 rows), then edge
# blocks (xblk 0 rows [16:96], xblk 7 rows [0:80])
MIDW = 6 * (256 + 128)     # 2304 cols, 96 rows
EDGW = 256 + 128           # 384 cols per edge block, 80 rows
HGW = MIDW + 2 * EDGW      # 3072 cols per hgrp (at mixed row counts)

BF16 = ml_dtypes.bfloat16


def _host_prep(features: np.ndarray, masks: np.ndarray):
    """Per-core packed (ftr || bnd) chunk operands.

    Returns fbm [96, NHG*MIDW] (mid blocks) and fbe [80, NHG*2*EDGW]
    (edge blocks, xblk0 then xblk7 per hgrp), both bf16.
    """
    featT = features.transpose(0, 2, 3, 1)  # [N, H, W, C]
    fbms, fbes = [], []
    for i in range(NCORES):
        n, yh = divmod(i, 2)
        # padded transposed features: feat_pad[hp, wp, c] = features[n, c, yh*32+hp-2, wp-2]
        feat_pad = np.zeros((HPL, WP, C), np.float32)
        r0 = yh * 32 - 2
        lo, hi = max(0, -r0), min(HPL, H - r0)
        feat_pad[lo:hi, 2:2 + W, :] = featT[n, r0 + lo:r0 + hi]

        # ftr[k=(wpw*8+hpw), t=(hgrp, xblk), c]
        s_hp, s_wp, s_c = feat_pad.strides
        ftr = np.lib.stride_tricks.as_strided(
            feat_pad,
            shape=(12, 8, NHG, NXB, C),
            strides=(s_wp, s_hp, 4 * s_hp, 8 * s_wp, s_c),
        ).reshape(KDIM, NHG, NXB, C)

        # bnd[k=(wpw,hpw), (hgrp, xblk), (p4, py, xl)]
        ml = masks[n, :, yh * 64:(yh + 1) * 64, :]  # [25, 64, 128]
        bnd = np.zeros((12, 8, NHG, NXB, 4, 2, 16), np.float32)
        s = bnd.strides
        for kr in range(KK):
            for dw in range(KK):
                # dest dims (p4, hgrp, xblk, xw, py, q):
                #   bnd[dw+xw, p4+kr, hgrp, xblk, p4, py, 2*xw+q]
                dv = np.lib.stride_tricks.as_strided(
                    bnd[dw, kr],
                    shape=(4, NHG, NXB, 8, 2, 2),
                    strides=(s[1] + s[4], s[2], s[3], s[0] + 2 * s[6], s[5], s[6]),
                )
                sv = ml[kr * KK + dw].reshape(NHG, 4, 2, NXB, 8, 2)
                dv[...] = sv.transpose(1, 0, 3, 4, 2, 5)
        bnd = bnd.reshape(KDIM, NHG, NXB, 128)

        fbm = np.empty((KDIM, NHG, MIDW), np.float32)
        fbm[:, :, :6 * C] = ftr[:, :, 1:7].reshape(KDIM, NHG, 6 * C)
        fbm[:, :, 6 * C:] = bnd[:, :, 1:7].reshape(KDIM, NHG, 6 * 128)
        fbe = np.empty((KE, NHG, 2, EDGW), np.float32)
        fbe[:, :, 0, :C] = ftr[16:, :, 0]
        fbe[:, :, 0, C:] = bnd[16:, :, 0]
        fbe[:, :, 1, :C] = ftr[:KE, :, 7]
        fbe[:, :, 1, C:] = bnd[:KE, :, 7]
        fbms.append(fbm.reshape(KDIM, NHG * MIDW).astype(BF16))
        fbes.append(fbe.reshape(KE, NHG * 2 * EDGW).astype(BF16))
    return fbms, fbes


_NC_CACHE = []


def _build_nc():
    """Build + compile the single-core Tile program (same for all 8 cores)."""
    if _NC_CACHE:
        return _NC_CACHE[0]

    nc = bacc.Bacc("TRN2", target_bir_lowering=False, debug=False)
    fbm = nc.dram_tensor(
        "fbm", [KDIM, NHG * MIDW], mybir.dt.bfloat16, kind="ExternalInput"
    ).ap()
    fbe = nc.dram_tensor(
        "fbe", [KE, NHG * 2 * EDGW], mybir.dt.bfloat16, kind="ExternalInput"
    ).ap()
    out = nc.dram_tensor(
        "out", [C, 64 * 128], mybir.dt.bfloat16, kind="ExternalOutput"
    ).ap()
    # out view [ch, c, hgrp, f=1024]
    ov = out.rearrange("(ch c) (hgrp f) -> ch c hgrp f", ch=2, hgrp=NHG)

    with tile.TileContext(nc) as tc:
        with (
            tc.tile_pool(name="fbp", bufs=2 * len(CHUNKS)) as fbp,
            tc.tile_pool(name="stp", bufs=8) as stp,
            tc.tile_pool(name="pp", bufs=2, space="PSUM") as pp,
        ):
            chunk_of = []   # per hgrp: (mid tile, edge tile, local hgrp idx)
            moff = eoff = 0
            for ci, g in enumerate(CHUNKS):
                tm = fbp.tile([KDIM, g * MIDW], mybir.dt.bfloat16,
                              name="fbm", tag="fbm")
                te = fbp.tile([KE, g * 2 * EDGW], mybir.dt.bfloat16,
                              name="fbe", tag="fbe")
                eng = nc.gpsimd if ci == 0 else nc.sync
                eng.dma_start(tm[:], fbm[:, moff:moff + g * MIDW])
                nc.sync.dma_start(te[:], fbe[:, eoff:eoff + g * 2 * EDGW])
                moff += g * MIDW
                eoff += g * 2 * EDGW
                for hh in range(g):
                    chunk_of.append((tm, te, hh))

            for hgrp in range(NHG):
                tm, te, hh = chunk_of[hgrp]
                ps = pp.tile([128, 2048], mybir.dt.float32, name="ps", tag="ps")
                for xblk in range(NXB):
                    if xblk in (0, 7):
                        e = 0 if xblk == 0 else 1
                        base = (hh * 2 + e) * EDGW
                        rhs = te[:, base + C: base + C + 128]
                        lhs = [te[:, base + ch * 128: base + (ch + 1) * 128]
                               for ch in range(2)]
                    else:
                        base = hh * MIDW + (xblk - 1) * C
                        bb = hh * MIDW + 6 * C + (xblk - 1) * 128
                        rhs = tm[:, bb: bb + 128]
                        lhs = [tm[:, base + ch * 128: base + (ch + 1) * 128]
                               for ch in range(2)]
                    for ch in range(2):
                        nc.tensor.matmul(
                            ps[:, (xblk * 2 + ch) * 128: (xblk * 2 + ch + 1) * 128],
                            lhs[ch],
                            rhs,
                            start=True,
                            stop=True,
                        )
                st = stp.tile([128, 2048], mybir.dt.bfloat16, name="st", tag="st")
                # psum cols (xblk, ch, p4, py, xl) -> staging cols (ch, p4, py, xblk, xl)
                sv = ps.rearrange(
                    "c (xblk ch p4 py xl) -> c ch xblk p4 py xl",
                    xblk=8, ch=2, p4=4, py=2,
                )
                dv = st.rearrange(
                    "c (ch p4 py xblk xl) -> c ch xblk p4 py xl",
                    ch=2, p4=4, py=2, xblk=8,
                )
                nc.vector.tensor_copy(dv[:, 0], sv[:, 0])
                nc.scalar.copy(dv[:, 1], sv[:, 1])
                # staging [c, (ch f)] -> out[ch*128+c, hgrp*1024 + f]
                sov = st.rearrange("c (ch f) -> c ch f", ch=2)
                nc.sync.dma_start(ov[:, :, hgrp, :].rearrange("ch c f -> c ch f"), sov)

    nc.compile()
    _NC_CACHE.append(nc)
    return nc


def kernel(features: np.ndarray, masks: np.ndarray) -> np.ndarray:
    features = np.ascontiguousarray(features, dtype=np.float32)
    masks = np.ascontiguousarray(masks, dtype=np.float32)
    fbms, fbes = _host_prep(features, masks)

    nc = _build_nc()
    in_maps = [{"fbm": fbms[i], "fbe": fbes[i]} for i in range(NCORES)]

    res = bass_utils.run_bass_kernel_spmd(nc, in_maps, list(range(NCORES)))

    out = np.empty((N, C, HO, WO), np.float32)
    for i in range(NCORES):
        n, yh = divmod(i, 2)
        out[n, :, yh * 64:(yh + 1) * 64, :] = (
            res.results[i]["out"].astype(np.float32).reshape(C, 64, 128)
        )
    return out
